# revision 16
# baseline (speedup 1.0000x reference)
"""Trainium2 Bass kernel for nn_AutoregressiveCDF (MADE + rational-quadratic
spline CDF, product over features).

Strategy: pure data-parallel over 8 NeuronCores (batch 16384 -> 8 x 2048),
weights replicated.  Per core:

- All matmul operands in bf16 (l2 impact ~2e-4, gate is 2e-2).
- Hidden units permuted by MADE degree (ascending) on the host, which makes
  the masked weight matrices block-triangular at 128 granularity: the
  residual-block GEMMs skip 6/16 blocks and the output GEMM skips 24/64
  k-passes (all exactly zero, no accuracy change).
- Phases run serially (trunk, then output GEMM + spline).  Interleaving
  them was tried and made things worse: TRN2 DVS power management
  throttles the clocks when several engines run hot concurrently
  (throttle-active time rose 320us -> 505us and every instruction
  stretched ~25-40%), so concurrency does not pay here; less total work
  does.  For the same reason the bulk spline ops stay on DVE/ACT and the
  GpSimd engine is kept nearly idle.
- Spline evaluated in the *normalized* domain: widths scaled by CFREE/Sw
  so each feature's edge span is exactly 1.0, making the chained running
  edge value at feature f equal f + local_edge.  A fused scan-compare
  custom DVE op yields the bin mask u in one pass (compare against x + f),
  and six fused scan-MAC ops produce the gathered spline parameters
  (prefix sums at the bin index via segment-boundary diffs).  The min-bin
  affine is folded into the scans, so no bin-index tensor, searchsorted
  gather, or edge tensor is ever materialized.  Broadcast normalize ops
  and the softplus diff run on the (otherwise idle) GpSimd engine.
"""

import numpy as np
import ml_dtypes
from contextlib import ExitStack

import concourse.bass as bass
import concourse.bacc as bacc
import concourse.tile as tile
from concourse import mybir
from concourse.bass_utils import run_bass_kernel_spmd

F32 = mybir.dt.float32
BF16 = mybir.dt.bfloat16

# problem sizes (hardcoded per contract)
B, F, H, C = 16384, 64, 512, 512
NB = 30
MULT = 3 * NB + 1            # 91
NBLOCKS = 3
NCORES = 8
MIN_BIN = 1e-3
MIN_DERIV = 1e-3
CFREE = float(1.0 - MIN_BIN * NB)         # softmax mass after min-bin affine
SCALE = float(np.float32(1.0 / np.sqrt(H)))
FH = F // 2                  # features per half (32)
WOH = FH * MULT              # 2912 W_out cols per half
KH = H // 128                # 4 hidden chunks
# k-chunks needed per 4-feature output group (degree-sorted hidden)
KSETS = [1, 1, 1, 1, 2, 2, 2, 2, 3, 3, 3, 3, 4, 4, 4, 4]

# knobs (test.py may override module globals before calling kernel())
MM_DT = BF16                 # PE dtype
TRACE = False
LAST_RESULTS = None          # BassKernelResults of the most recent run

_CACHE = {}


def _masks():
    d_in = np.arange(1, F + 1)
    d_h = np.arange(H) % max(1, F - 1) + min(1, F - 1)
    m_in = (d_h[None, :] >= d_in[:, None]).astype(np.float32)
    m_hh = (d_h[None, :] >= d_h[:, None]).astype(np.float32)
    d_out = np.repeat(d_in, MULT)
    m_out = (d_out[None, :] > d_h[:, None]).astype(np.float32)
    return m_in, m_hh, m_out, d_h


def _scanmac_ref(in0, in1, s0, s1, imm2):
    a = np.asarray(in0, np.float32).reshape(np.asarray(in0).shape[0], -1)
    b = np.asarray(in1, np.float32).reshape(a.shape)
    return np.cumsum(a * (b + np.float32(s0)), axis=1,
                     dtype=np.float32).reshape(np.asarray(in0).shape)


def _scancmp_ref(in0, in1, s0, s1, imm2):
    a = np.asarray(in0, np.float32).reshape(np.asarray(in0).shape[0], -1)
    t = np.asarray(in1, np.float32).reshape(a.shape)
    s = np.cumsum(a + np.float32(s0), axis=1, dtype=np.float32)
    return (t >= s).astype(np.float32).reshape(np.asarray(in0).shape)


def _register_spline_ops():
    """SCAN_MAC_ANT: out = cumsum(in0 * (in1 + s0))   (chained masked MAC)
    SCANCMP_ANT:  out = (in1 >= cumsum(in0 + s0))  (bin-search mask)"""
    import concourse.dve_ops as dve_ops
    from concourse.dve_spec import Spec, Src0, Src1, C0, scan, AluOp, lower
    from concourse.dve_uop import DveOpSpec
    have = {op.name: op for op in dve_ops.OPS}
    if "SCAN_MAC_ANT" in have and "SCANCMP_ANT" in have:
        return have["SCAN_MAC_ANT"], have["SCANCMP_ANT"]

    def reg(name, spec):
        row = max(dve_ops._SUB_OPCODE_FOR_NAME.values()) + 1
        assert row < 0x20
        shas = {}
        for ver in ("v3", "v4"):
            u = lower(spec, ver=ver)
            shas[ver] = DveOpSpec(name=name, opcode=row, uops=u,
                                  rd1_en=True).sha(ver)
        op = dve_ops.DveOp(name, spec, subdim=False, uops_sha=shas)
        dve_ops.OPS.append(op)
        dve_ops.CUSTOM_DVE_SPECS[name] = spec
        dve_ops._SUB_OPCODE_FOR_NAME[name] = row
        return op

    mac = reg("SCAN_MAC_ANT",
              Spec(body=scan(AluOp.ADD, Src0 * (Src1 + C0)),
                   reference=_scanmac_ref))
    cmp_ = reg("SCANCMP_ANT",
               Spec(body=scan(AluOp.ADD, Src0 + C0) <= Src1,
                    reference=_scancmp_ref))
    return mac, cmp_


class _Bacc(bacc.Bacc):
    """Bacc with a trimmed activation-table list so Exp and Ln share one
    table (no per-chunk ACT_TABLE_LOAD thrash)."""

    _KEEP_TABLES = ("natural_log_exp_and_others", "sigmoid_and_others")

    def insert_act_table_loads(self):
        import bass_rust as _bass_rust
        from concourse.hw_specs import get_activation_tables
        import concourse.mybir as _mb
        has_activation = any(
            isinstance(i, _mb.InstActivation)
            for b in self.main_func.blocks
            for i in b.instructions
        )
        if not has_activation:
            return
        all_tables = get_activation_tables(self.m.arch)
        tables = [(k, (v if k in self._KEEP_TABLES else set()))
                  for k, v in all_tables.items()]
        _bass_rust.insert_act_table_loads(self, tables)


def _build(bc, mm_dt):
    """Build the per-core Bass module for bc batch rows per core."""
    nch = bc // 128          # 16 chunks of 128 batch rows
    nslices = bc // 512      # 4 slices of 512
    MMT = mm_dt
    scan_mac, scancmp = _register_spline_ops()
    nc = _Bacc("TRN2", target_bir_lowering=False, debug=False,
               enable_asserts=False)

    def din(name, shape, dt=F32):
        return nc.dram_tensor(name, list(shape), dt, kind="ExternalInput").ap()

    pred = din("pred", (bc, F))
    ctxm = din("ctx", (bc, C))
    w_in = din("w_in", (F, H), MMT)
    wc_in = din("wc_in", (C, H), MMT)
    wb1 = din("wb1", (NBLOCKS, H, H), MMT)
    wb2 = din("wb2", (NBLOCKS, H, H), MMT)
    wcb = din("wcb", (NBLOCKS, C, H), MMT)
    w_out = din("w_out", (H, F * MULT), MMT)
    b1 = din("b1", (H,))
    bb1 = din("bb1", (NBLOCKS, H))
    bb2 = din("bb2", (NBLOCKS, H))
    bcb = din("bcb", (NBLOCKS, H))
    ident = din("ident", (128, 128))
    fcon = din("fcon", (FH,))
    out_d = nc.dram_tensor("out", [bc], F32, kind="ExternalOutput").ap()

    AX = mybir.AxisListType
    OP = mybir.AluOpType
    ACTF = mybir.ActivationFunctionType

    def bcast(ap2d, n):
        return bass.AP(tensor=ap2d.tensor, offset=ap2d.offset,
                       ap=list(ap2d.ap) + [[0, n]])

    def pbcast(ap1d, p, n):
        return bass.AP(tensor=ap1d.tensor, offset=ap1d.offset,
                       ap=[[0, p]] + list(ap1d.ap))

    with tile.TileContext(nc) as tc, ExitStack() as ctx:
        const = ctx.enter_context(tc.tile_pool(name="const", bufs=1))
        persist = ctx.enter_context(tc.tile_pool(name="persist", bufs=1))
        wpool = ctx.enter_context(tc.tile_pool(name="wpool", bufs=1))
        apool = ctx.enter_context(tc.tile_pool(name="apool", bufs=1))
        pat = ctx.enter_context(tc.tile_pool(name="pat", bufs=1))

        TS = nc.vector.tensor_scalar
        TT = nc.vector.tensor_tensor
        STT = nc.vector.scalar_tensor_tensor

        def tscopy(dst, srcap):
            TS(out=dst, in0=srcap, scalar1=0.0, scalar2=None, op0=OP.add)

        ident_t = const.tile([128, 128], F32)
        nc.sync.dma_start(out=ident_t[:], in_=ident)
        fc_t = const.tile([128, FH], F32)
        nc.sync.dma_start(out=fc_t[:], in_=pbcast(fcon, 128, FH))
        one_t = const.tile([128, 1], F32)
        nc.vector.memset(one_t[:], 1.0)
        b1_t = const.tile([128, KH], F32)
        nc.sync.dma_start(out=b1_t[:], in_=b1.rearrange("(m p) -> p m", p=128))
        bb1_t = const.tile([128, NBLOCKS, KH], F32)
        bb2_t = const.tile([128, NBLOCKS, KH], F32)
        bcb_t = const.tile([128, NBLOCKS, KH], F32)
        for tt_, src in ((bb1_t, bb1), (bb2_t, bb2), (bcb_t, bcb)):
            nc.sync.dma_start(out=tt_[:],
                              in_=src.rearrange("i (m p) -> p i m", p=128))

        # persistent activations / outputs
        ctx_T = [apool.tile([128, bc], MMT, tag=f"ctxT{k}", name=f"ctxT{k}")
                 for k in range(KH)]
        x_T = apool.tile([64, bc], MMT)
        t_t = [apool.tile([128, bc], MMT, tag=f"t{k}", name=f"t{k}")
               for k in range(KH)]
        halfprod = persist.tile([128, nch, 2], F32)


        # -------- transposes of pred/ctx for the whole core, up front ------
        with tc.tile_pool(name="pst", bufs=2, space="PSUM") as pst, \
             tc.tile_pool(name="ldp", bufs=2) as ldp:
            for c in range(nch):
                ld = ldp.tile([128, C], F32, tag="ctxld", name="ctxld")
                nc.sync.dma_start(out=ld[:], in_=ctxm[c * 128:(c + 1) * 128, :])
                for k in range(KH):
                    ps = pst.tile([128, 128], F32, tag="tp", name="tp")
                    nc.tensor.transpose(ps[:], ld[:, k * 128:(k + 1) * 128],
                                        ident_t[:])
                    nc.scalar.activation(out=ctx_T[k][:, c * 128:(c + 1) * 128],
                                         in_=ps[:], func=ACTF.Copy)
                pld = ldp.tile([128, F], F32, tag="predld", name="predld")
                nc.sync.dma_start(out=pld[:], in_=pred[c * 128:(c + 1) * 128, :])
                ps = pst.tile([64, 128], F32, tag="tpp", name="tpp")
                nc.tensor.transpose(ps[:], pld[:], ident_t[:])
                nc.scalar.activation(out=x_T[:, c * 128:(c + 1) * 128],
                                     in_=ps[:], func=ACTF.Copy)

        # trunk weights, all resident (bf16)
        w_in_t = wpool.tile([64, H], MMT)
        nc.sync.dma_start(out=w_in_t[:], in_=w_in)
        wc_in_t = [wpool.tile([128, H], MMT, tag=f"wci{k}", name=f"wci{k}")
                   for k in range(KH)]
        for k in range(KH):
            nc.sync.dma_start(out=wc_in_t[k][:],
                              in_=wc_in[k * 128:(k + 1) * 128, :])
        wstream = ctx.enter_context(tc.tile_pool(name="wstream", bufs=2))

        def load_block_w(i):
            wbt = {}
            for nm, src in (("wb1", wb1), ("wb2", wb2), ("wcb", wcb)):
                for k in range(KH):
                    t_ = wstream.tile([128, H], MMT, tag=f"{nm}_{k}",
                                      name=f"{nm}_{k}")
                    nc.sync.dma_start(out=t_[:],
                                      in_=src[i, k * 128:(k + 1) * 128, :])
                    wbt[(nm, i, k)] = t_
            return wbt
        # W_out: one feature-half at a time (ring reuse via same tags)
        wo_t = [wpool.tile([128, WOH], MMT, tag=f"wo{k}", name=f"wo{k}")
                for k in range(KH)]

        def load_wo(half):
            for k in range(KH):
                nc.sync.dma_start(
                    out=wo_t[k][:],
                    in_=w_out[k * 128:(k + 1) * 128,
                              half * WOH:(half + 1) * WOH])

        load_wo(0)

        psm = ctx.enter_context(tc.tile_pool(name="psm", bufs=2, space="PSUM"))
        spl = ctx.enter_context(tc.tile_pool(name="spl", bufs=2))
        grp = ctx.enter_context(tc.tile_pool(name="grp", bufs=1))

        def ps4():
            return psm.tile([128, 4, 512], F32, tag="ps4", name="ps4")

        # ---------------- phase-B chunk processing ----------------
        GRP = 8
        gtiles = {}

        def gt(nm):
            if nm not in gtiles:
                gtiles[nm] = grp.tile([128, GRP, FH], F32, tag=nm, name=nm)
            return gtiles[nm]

        gRall = grp.tile([128, GRP, 6, FH], F32, tag="gRall", name="gRall")

        def b_chunk(c, half, gi):
            """Output GEMM + spline front for batch chunk c, feature half."""
            csl = slice(c * 128, (c + 1) * 128)
            gX = gt("gX")
            nc.sync.dma_start(out=gX[:, gi, :],
                              in_=pred[csl, half * FH:(half + 1) * FH])
            EWH = spl.tile([128, 2, FH, NB], F32, tag="EWH", name="EWH")
            EW = EWH[:, 0]
            EH = EWH[:, 1]
            ED = spl.tile([128, FH, NB + 1], F32, tag="ED", name="ED",
                          bufs=1)
            for n in range(2):
                ps = ps4()
                for j in range(4):
                    gg = half * 8 + n * 4 + j
                    nk = KSETS[gg]
                    nsl = slice((n * 4 + j) * 364, (n * 4 + j + 1) * 364)
                    for k in range(nk):
                        nc.tensor.matmul(ps[:, j, 0:364],
                                         t_t[k][:, csl],
                                         wo_t[k][:, nsl],
                                         start=(k == 0), stop=(k == nk - 1))
                psv = bass.AP(tensor=ps[:].tensor, offset=ps[:].offset,
                              ap=[ps[:].ap[0], [512, 4], [MULT, 4], [1, MULT]])
                fsl = slice(n * 16, (n + 1) * 16)
                nc.scalar.activation(
                    out=EW[:, fsl, :].rearrange("p (a f) n -> p a f n", a=4),
                    in_=psv[:, :, :, 0:NB], func=ACTF.Exp, scale=SCALE)
                nc.scalar.activation(
                    out=EH[:, fsl, :].rearrange("p (a f) n -> p a f n", a=4),
                    in_=psv[:, :, :, NB:2 * NB], func=ACTF.Exp, scale=SCALE)
                nc.scalar.activation(
                    out=ED[:, fsl, :].rearrange("p (a f) n -> p a f n", a=4),
                    in_=psv[:, :, :, 2 * NB:MULT], func=ACTF.Exp)
            # D = softplus(ud) = ln(exp(ud) + 1), in place over ED
            D = ED
            nc.scalar.activation(out=D[:].rearrange("p f n -> p (f n)"),
                                 in_=ED[:].rearrange("p f n -> p (f n)"),
                                 func=ACTF.Ln, bias=one_t[:])
            # per-feature sums, both tables in one reduce
            Sw = spl.tile([128, 2, FH], F32, tag="Sw", name="Sw")
            nc.vector.tensor_reduce(out=Sw[:], in_=EWH[:],
                                    axis=AX.X, op=OP.add)
            CRb = spl.tile([128, 2, FH], F32, tag="CRb", name="CRb", bufs=1)
            nc.vector.reciprocal(out=CRb[:].rearrange("p a f -> p (a f)"),
                                 in_=Sw[:].rearrange("p a f -> p (a f)"))
            TS(out=CRb[:].rearrange("p a f -> p (a f)"),
               in0=CRb[:].rearrange("p a f -> p (a f)"),
               scalar1=CFREE, scalar2=None, op0=OP.mult)
            # normalized widths/heights + softplus diffs on GpSimd
            EWHn = spl.tile([128, 2, FH, NB], F32, tag="EWHn", name="EWHn",
                            bufs=1)
            TT(out=EWHn[:], in0=EWH[:],
               in1=bass.AP(tensor=CRb[:].tensor, offset=CRb[:].offset,
                           ap=list(CRb[:].ap) + [[0, NB]]), op=OP.mult)
            EWn = EWHn[:, 0]
            EHn = EWHn[:, 1]
            dd = spl.tile([128, FH, NB], F32, tag="dd", name="dd", bufs=1)
            TT(out=dd[:], in0=D[:, :, 1:NB + 1], in1=D[:, :, 0:NB],
               op=OP.subtract)
            # bin-search mask in one fused scan-compare
            xpf = spl.tile([128, FH], F32, tag="xpf", name="xpf", bufs=1)
            TT(out=xpf[:], in0=gX[:, gi, :], in1=fc_t[:], op=OP.add)
            u = spl.tile([128, FH, NB], F32, tag="u", name="u", bufs=1)
            nc.vector._custom_dve(scancmp, out=u[:], in0=EWn,
                                  in1=bcast(xpf[:], NB), s0=MIN_BIN)
            # six fused masked-MAC gathers (chained; diff at segment ends)
            Rbig = spl.tile([128, 6, FH, NB - 1], F32, tag="Rbig",
                            name="Rbig", bufs=1)
            u0 = u[:, :, 0:NB - 1]
            streams = ((EWn[:, :, 0:NB - 1], MIN_BIN),
                       (EWn[:, :, 1:NB], MIN_BIN),
                       (EHn[:, :, 0:NB - 1], MIN_BIN),
                       (EHn[:, :, 1:NB], MIN_BIN),
                       (dd[:, :, 0:NB - 1], 0.0),
                       (dd[:, :, 1:NB], 0.0))
            for i_s, (t_in1, imm) in enumerate(streams):
                nc.vector._custom_dve(scan_mac, out=Rbig[:, i_s, :, :],
                                      in0=u0, in1=t_in1, s0=imm)
            Rl6 = bass.AP(tensor=Rbig[:].tensor,
                          offset=Rbig[:].offset + NB - 2,
                          ap=[Rbig[:].ap[0], [FH * (NB - 1), 6], [NB - 1, FH]])
            tscopy(gRall[:, gi, :, :], Rl6)
            # first-element extracts (ACT, strided)
            nc.scalar.activation(
                out=gt("gEWn0")[:, gi, :],
                in_=bass.AP(tensor=EWn.tensor, offset=EWn.offset,
                            ap=[EWn.ap[0], [NB, FH]]), func=ACTF.Copy)
            nc.scalar.activation(
                out=gt("gEHn0")[:, gi, :],
                in_=bass.AP(tensor=EHn.tensor, offset=EHn.offset,
                            ap=[EHn.ap[0], [NB, FH]]), func=ACTF.Copy)
            nc.scalar.activation(
                out=gt("gD0")[:, gi, :],
                in_=bass.AP(tensor=D[:].tensor, offset=D[:].offset,
                            ap=[D[:].ap[0], [NB + 1, FH]]), func=ACTF.Copy)
            nc.scalar.activation(
                out=gt("gD1")[:, gi, :],
                in_=bass.AP(tensor=D[:].tensor, offset=D[:].offset + 1,
                            ap=[D[:].ap[0], [NB + 1, FH]]), func=ACTF.Copy)

        def b_group_chain(gidx, half):
            """Finish the spline for GRP chunks on [128, GRP, FH] tiles."""
            def g2t(nm):
                return grp.tile([128, GRP, FH], F32, tag=nm, name=nm, bufs=1)
            gX = gt("gX")
            gdall = grp.tile([128, GRP, 6, FH], F32, tag="gdall",
                             name="gdall", bufs=1)
            TT(out=gdall[:, :, :, 1:FH], in0=gRall[:, :, :, 1:FH],
               in1=gRall[:, :, :, 0:FH - 1], op=OP.subtract)
            tscopy(gdall[:, :, :, 0:1], gRall[:, :, :, 0:1])
            s1d = gdall[:, :, 0, :]   # in_cw (normalized left edge)
            s2d = gdall[:, :, 1, :]
            s3d = gdall[:, :, 2, :]   # in_ch
            s4d = gdall[:, :, 3, :]
            s5d = gdall[:, :, 4, :]   # D_idx - D_0
            s6d = gdall[:, :, 5, :]   # D_{idx+1} - D_1
            inw = g2t("inw")
            TT(out=inw[:], in0=s2d, in1=s1d, op=OP.subtract)
            STT(out=inw[:], in0=inw[:], scalar=MIN_BIN, in1=gt("gEWn0")[:],
                op0=OP.add, op1=OP.add)
            rw = g2t("rw")
            nc.vector.reciprocal(out=rw[:], in_=inw[:])
            th = g2t("th")
            TT(out=th[:], in0=gX[:], in1=s1d, op=OP.subtract)
            TT(out=th[:], in0=th[:], in1=rw[:], op=OP.mult)
            inh = g2t("inh")
            TT(out=inh[:], in0=s4d, in1=s3d, op=OP.subtract)
            STT(out=inh[:], in0=inh[:], scalar=MIN_BIN, in1=gt("gEHn0")[:],
                op0=OP.add, op1=OP.add)
            dl = g2t("dl")
            TT(out=dl[:], in0=inh[:], in1=rw[:], op=OP.mult)
            ind = g2t("ind")
            STT(out=ind[:], in0=s5d, scalar=MIN_DERIV, in1=gt("gD0")[:],
                op0=OP.add, op1=OP.add)
            indp = g2t("indp")
            STT(out=indp[:], in0=s6d, scalar=MIN_DERIV, in1=gt("gD1")[:],
                op0=OP.add, op1=OP.add)
            om = g2t("tA")
            TS(out=om[:], in0=th[:], scalar1=-1.0, scalar2=1.0,
               op0=OP.mult, op1=OP.add)
            ttv = g2t("ttv")
            TT(out=ttv[:], in0=th[:], in1=om[:], op=OP.mult)
            th2 = g2t("tA")
            TT(out=th2[:], in0=th[:], in1=th[:], op=OP.mult)
            na = g2t("na")
            TT(out=na[:], in0=dl[:], in1=th2[:], op=OP.mult)
            nb_ = g2t("tA")
            TT(out=nb_[:], in0=ind[:], in1=ttv[:], op=OP.mult)
            TT(out=na[:], in0=na[:], in1=nb_[:], op=OP.add)
            TT(out=na[:], in0=na[:], in1=inh[:], op=OP.mult)
            s1_ = g2t("s1_")
            TT(out=s1_[:], in0=ind[:], in1=indp[:], op=OP.add)
            STT(out=s1_[:], in0=dl[:], scalar=-2.0, in1=s1_[:],
                op0=OP.mult, op1=OP.add)
            TT(out=s1_[:], in0=s1_[:], in1=ttv[:], op=OP.mult)
            TT(out=s1_[:], in0=s1_[:], in1=dl[:], op=OP.add)
            rden = g2t("tA")
            nc.vector.reciprocal(out=rden[:], in_=s1_[:])
            cdf = na
            TT(out=cdf[:], in0=na[:], in1=rden[:], op=OP.mult)
            TT(out=cdf[:], in0=cdf[:], in1=s3d, op=OP.add)
            hp = halfprod[:, gidx * GRP:(gidx + 1) * GRP, half:half + 1]
            nc.vector.tensor_reduce(
                out=hp.rearrange("p g h -> p (g h)"), in_=cdf[:],
                axis=AX.X, op=OP.mult)

        # ---------------- phase A: MADE trunk (serial) ----------------
        for s in range(nslices):
            bsl = slice(s * 512, (s + 1) * 512)
            ps = ps4()
            for m in range(KH):
                nc.tensor.matmul(ps[:, m, :], w_in_t[:, m * 128:(m + 1) * 128],
                                 x_T[:, bsl], start=True, stop=False)
                for k in range(KH):
                    nc.tensor.matmul(ps[:, m, :],
                                     wc_in_t[k][:, m * 128:(m + 1) * 128],
                                     ctx_T[k][:, bsl],
                                     start=False, stop=(k == KH - 1))
            for m in range(KH):
                nc.scalar.activation(out=t_t[m][:, bsl], in_=ps[:, m, :],
                                     func=ACTF.Identity, bias=b1_t[:, m:m + 1])
        # residual blocks (lower-triangular wb1/wb2 blocks are zero)
        for i in range(NBLOCKS):
            wbt = load_block_w(i)
            for s in range(nslices):
                bsl = slice(s * 512, (s + 1) * 512)
                h1t = pat.tile([128, KH, 512], MMT, tag="h1t", name="h1t")
                for k in range(KH):
                    nc.scalar.activation(out=h1t[:, k, :], in_=t_t[k][:, bsl],
                                         func=ACTF.Relu)
                ps1 = ps4()
                for m in range(KH):
                    for k in range(m + 1):
                        nc.tensor.matmul(ps1[:, m, :],
                                         wbt[("wb1", i, k)][:, m * 128:(m + 1) * 128],
                                         h1t[:, k, :],
                                         start=(k == 0), stop=(k == m))
                h2t = pat.tile([128, KH, 512], MMT, tag="h2t", name="h2t")
                for m in range(KH):
                    nc.scalar.activation(out=h2t[:, m, :], in_=ps1[:, m, :],
                                         func=ACTF.Relu,
                                         bias=bb1_t[:, i, m:m + 1])
                ps2 = ps4()
                for m in range(KH):
                    for k in range(m + 1):
                        nc.tensor.matmul(ps2[:, m, :],
                                         wbt[("wb2", i, k)][:, m * 128:(m + 1) * 128],
                                         h2t[:, k, :],
                                         start=(k == 0), stop=(k == m))
                ps3 = ps4()
                for m in range(KH):
                    for k in range(KH):
                        nc.tensor.matmul(ps3[:, m, :],
                                         wbt[("wcb", i, k)][:, m * 128:(m + 1) * 128],
                                         ctx_T[k][:, bsl],
                                         start=(k == 0), stop=(k == KH - 1))
                for m in range(KH):
                    g_ = pat.tile([128, 512], F32, tag="g", name="g", bufs=2)
                    nc.scalar.activation(out=g_[:], in_=ps3[:, m, :],
                                         func=ACTF.Sigmoid,
                                         bias=bcb_t[:, i, m:m + 1])
                    v = pat.tile([128, 512], F32, tag="v", name="v", bufs=2)
                    STT(out=v[:], in0=ps2[:, m, :],
                        scalar=bb2_t[:, i, m:m + 1], in1=g_[:],
                        op0=OP.add, op1=OP.mult)
                    nc.gpsimd.tensor_tensor(out=t_t[m][:, bsl],
                                            in0=t_t[m][:, bsl], in1=v[:],
                                            op=OP.add)

        # ---------------- phase B: output GEMM + spline ----------------
        for half in range(2):
            if half == 1:
                load_wo(1)
            for c in range(nch):
                b_chunk(c, half, c % GRP)
                if c % GRP == GRP - 1:
                    b_group_chain(c // GRP, half)

        fp = persist.tile([128, nch], F32)
        nc.vector.tensor_tensor(
            out=fp[:],
            in0=halfprod[:, :, 0:1].rearrange("p c h -> p (c h)"),
            in1=halfprod[:, :, 1:2].rearrange("p c h -> p (c h)"),
            op=OP.mult)
        nc.sync.dma_start(out=out_d.rearrange("(c p) -> p c", p=128),
                          in_=fp[:])

    nc.compile()
    return nc


def _prep_shared(W_in, b_in, Wc_in, bc_in, Wb1, bb1, Wb2, bb2, Wcb, bcb,
                 W_out, b_out, mm_dt):
    m_in, m_hh, m_out, d_h = _masks()
    assert not np.any(b_out), "nonzero b_out not supported by this kernel"
    perm = np.argsort(d_h, kind="stable")
    if mm_dt == BF16:
        wdt = ml_dtypes.bfloat16
    else:
        wdt = np.float32
    def rnd(a):
        return np.ascontiguousarray(np.asarray(a, np.float32).astype(wdt))
    shared = {
        "w_in": rnd((W_in * m_in)[:, perm]),
        "wc_in": rnd(np.asarray(Wc_in)[:, perm]),
        "wb1": rnd((Wb1 * m_hh[None])[:, perm][:, :, perm]),
        "wb2": rnd((Wb2 * m_hh[None])[:, perm][:, :, perm]),
        "wcb": rnd(np.asarray(Wcb)[:, :, perm]),
        "w_out": rnd((W_out * m_out)[perm, :]),
        "b1": np.ascontiguousarray((b_in + bc_in).astype(np.float32)[perm]),
        "bb1": np.ascontiguousarray(np.asarray(bb1, np.float32)[:, perm]),
        "bb2": np.ascontiguousarray(np.asarray(bb2, np.float32)[:, perm]),
        "bcb": np.ascontiguousarray(np.asarray(bcb, np.float32)[:, perm]),
        "ident": np.eye(128, dtype=np.float32),
        "fcon": np.arange(FH, dtype=np.float32),
    }
    return shared


def kernel(predicates, contexts, W_in, b_in, Wc_in, bc_in, Wb1, bb1, Wb2, bb2,
           Wcb, bcb, W_out, b_out):
    global LAST_RESULTS
    predicates = np.asarray(predicates, dtype=np.float32)
    contexts = np.asarray(contexts, dtype=np.float32)
    bc = predicates.shape[0] // NCORES
    key = (bc, MM_DT)
    if key not in _CACHE:
        _CACHE[key] = _build(bc, MM_DT)
    nc = _CACHE[key]
    shared = _prep_shared(W_in, b_in, Wc_in, bc_in, Wb1, bb1, Wb2, bb2,
                          Wcb, bcb, W_out, b_out, MM_DT)
    in_maps = []
    for cid in range(NCORES):
        sl = slice(cid * bc, (cid + 1) * bc)
        m = dict(shared)
        m["pred"] = np.ascontiguousarray(predicates[sl])
        m["ctx"] = np.ascontiguousarray(contexts[sl])
        in_maps.append(m)
    res = run_bass_kernel_spmd(nc, in_maps, core_ids=list(range(NCORES)),
                               trace=TRACE)
    LAST_RESULTS = res
    return np.concatenate([res.results[i]["out"] for i in range(NCORES)])


# revision 17
# speedup vs baseline: 1.0039x; 1.0039x over previous
"""Trainium2 Bass kernel for nn_AutoregressiveCDF (MADE + rational-quadratic
spline CDF, product over features).

Strategy: pure data-parallel over 8 NeuronCores (batch 16384 -> 8 x 2048),
weights replicated.  Per core:

- All matmul operands in bf16 (l2 impact ~2e-4, gate is 2e-2).
- Hidden units permuted by MADE degree (ascending) on the host, which makes
  the masked weight matrices block-triangular at 128 granularity: the
  residual-block GEMMs skip 6/16 blocks and the output GEMM skips 24/64
  k-passes (all exactly zero, no accuracy change).
- Phases run serially (trunk, then output GEMM + spline).  Interleaving
  them was tried and made things worse: TRN2 DVS power management
  throttles the clocks when several engines run hot concurrently
  (throttle-active time rose 320us -> 505us and every instruction
  stretched ~25-40%), so concurrency does not pay here; less total work
  does.  For the same reason the bulk spline ops stay on DVE/ACT and the
  GpSimd engine is kept nearly idle.
- Spline evaluated in the *normalized* domain: widths scaled by CFREE/Sw
  so each feature's edge span is exactly 1.0, making the chained running
  edge value at feature f equal f + local_edge.  A fused scan-compare
  custom DVE op yields the bin mask u in one pass (compare against x + f),
  and six fused scan-MAC ops produce the gathered spline parameters
  (prefix sums at the bin index via segment-boundary diffs).  The min-bin
  affine is folded into the scans, so no bin-index tensor, searchsorted
  gather, or edge tensor is ever materialized.  Broadcast normalize ops
  and the softplus diff run on the (otherwise idle) GpSimd engine.
"""

import numpy as np
import ml_dtypes
from contextlib import ExitStack

import concourse.bass as bass
import concourse.bacc as bacc
import concourse.tile as tile
from concourse import mybir
from concourse.bass_utils import run_bass_kernel_spmd

F32 = mybir.dt.float32
BF16 = mybir.dt.bfloat16
FP8 = mybir.dt.float8e4

# problem sizes (hardcoded per contract)
B, F, H, C = 16384, 64, 512, 512
NB = 30
MULT = 3 * NB + 1            # 91
NBLOCKS = 3
NCORES = 8
MIN_BIN = 1e-3
MIN_DERIV = 1e-3
CFREE = float(1.0 - MIN_BIN * NB)         # softmax mass after min-bin affine
SCALE = float(np.float32(1.0 / np.sqrt(H)))
FH = F // 2                  # features per half (32)
WOH = FH * MULT              # 2912 W_out cols per half
KH = H // 128                # 4 hidden chunks
# k-chunks needed per 4-feature output group (degree-sorted hidden)
KSETS = [1, 1, 1, 1, 2, 2, 2, 2, 3, 3, 3, 3, 4, 4, 4, 4]

# knobs (test.py may override module globals before calling kernel())
MM_DT = BF16                 # PE dtype
TRACE = False
LAST_RESULTS = None          # BassKernelResults of the most recent run

_CACHE = {}


def _masks():
    d_in = np.arange(1, F + 1)
    d_h = np.arange(H) % max(1, F - 1) + min(1, F - 1)
    m_in = (d_h[None, :] >= d_in[:, None]).astype(np.float32)
    m_hh = (d_h[None, :] >= d_h[:, None]).astype(np.float32)
    d_out = np.repeat(d_in, MULT)
    m_out = (d_out[None, :] > d_h[:, None]).astype(np.float32)
    return m_in, m_hh, m_out, d_h


def _scanmac_ref(in0, in1, s0, s1, imm2):
    a = np.asarray(in0, np.float32).reshape(np.asarray(in0).shape[0], -1)
    b = np.asarray(in1, np.float32).reshape(a.shape)
    return np.cumsum(a * (b + np.float32(s0)), axis=1,
                     dtype=np.float32).reshape(np.asarray(in0).shape)


def _scancmp_ref(in0, in1, s0, s1, imm2):
    a = np.asarray(in0, np.float32).reshape(np.asarray(in0).shape[0], -1)
    t = np.asarray(in1, np.float32).reshape(a.shape)
    s = np.cumsum(a + np.float32(s0), axis=1, dtype=np.float32)
    return (t >= s).astype(np.float32).reshape(np.asarray(in0).shape)


def _register_spline_ops():
    """SCAN_MAC_ANT: out = cumsum(in0 * (in1 + s0))   (chained masked MAC)
    SCANCMP_ANT:  out = (in1 >= cumsum(in0 + s0))  (bin-search mask)"""
    import concourse.dve_ops as dve_ops
    from concourse.dve_spec import Spec, Src0, Src1, C0, scan, AluOp, lower
    from concourse.dve_uop import DveOpSpec
    have = {op.name: op for op in dve_ops.OPS}
    if "SCAN_MAC_ANT" in have and "SCANCMP_ANT" in have:
        return have["SCAN_MAC_ANT"], have["SCANCMP_ANT"]

    def reg(name, spec):
        row = max(dve_ops._SUB_OPCODE_FOR_NAME.values()) + 1
        assert row < 0x20
        shas = {}
        for ver in ("v3", "v4"):
            u = lower(spec, ver=ver)
            shas[ver] = DveOpSpec(name=name, opcode=row, uops=u,
                                  rd1_en=True).sha(ver)
        op = dve_ops.DveOp(name, spec, subdim=False, uops_sha=shas)
        dve_ops.OPS.append(op)
        dve_ops.CUSTOM_DVE_SPECS[name] = spec
        dve_ops._SUB_OPCODE_FOR_NAME[name] = row
        return op

    mac = reg("SCAN_MAC_ANT",
              Spec(body=scan(AluOp.ADD, Src0 * (Src1 + C0)),
                   reference=_scanmac_ref))
    cmp_ = reg("SCANCMP_ANT",
               Spec(body=scan(AluOp.ADD, Src0 + C0) <= Src1,
                    reference=_scancmp_ref))
    return mac, cmp_


class _Bacc(bacc.Bacc):
    """Bacc with a trimmed activation-table list so Exp and Ln share one
    table (no per-chunk ACT_TABLE_LOAD thrash)."""

    _KEEP_TABLES = ("natural_log_exp_and_others", "sigmoid_and_others")

    def insert_act_table_loads(self):
        import bass_rust as _bass_rust
        from concourse.hw_specs import get_activation_tables
        import concourse.mybir as _mb
        has_activation = any(
            isinstance(i, _mb.InstActivation)
            for b in self.main_func.blocks
            for i in b.instructions
        )
        if not has_activation:
            return
        all_tables = get_activation_tables(self.m.arch)
        tables = [(k, (v if k in self._KEEP_TABLES else set()))
                  for k, v in all_tables.items()]
        _bass_rust.insert_act_table_loads(self, tables)


def _build(bc, mm_dt):
    """Build the per-core Bass module for bc batch rows per core."""
    nch = bc // 128          # 16 chunks of 128 batch rows
    nslices = bc // 512      # 4 slices of 512
    MMT = mm_dt
    scan_mac, scancmp = _register_spline_ops()
    nc = _Bacc("TRN2", target_bir_lowering=False, debug=False,
               enable_asserts=False)

    def din(name, shape, dt=F32):
        return nc.dram_tensor(name, list(shape), dt, kind="ExternalInput").ap()

    pred = din("pred", (bc, F))
    ctxm = din("ctx", (bc, C))
    w_in = din("w_in", (F, H), FP8)
    wc_in = din("wc_in", (C, H), FP8)
    wb1 = din("wb1", (NBLOCKS, H, H), FP8)
    wb2 = din("wb2", (NBLOCKS, H, H), FP8)
    wcb = din("wcb", (NBLOCKS, C, H), FP8)
    w_out = din("w_out", (H, F * MULT), MMT)
    b1 = din("b1", (H,))
    bb1 = din("bb1", (NBLOCKS, H))
    bb2 = din("bb2", (NBLOCKS, H))
    bcb = din("bcb", (NBLOCKS, H))
    ident = din("ident", (128, 128))
    fcon = din("fcon", (FH,))
    out_d = nc.dram_tensor("out", [bc], F32, kind="ExternalOutput").ap()

    AX = mybir.AxisListType
    OP = mybir.AluOpType
    ACTF = mybir.ActivationFunctionType

    def bcast(ap2d, n):
        return bass.AP(tensor=ap2d.tensor, offset=ap2d.offset,
                       ap=list(ap2d.ap) + [[0, n]])

    def pbcast(ap1d, p, n):
        return bass.AP(tensor=ap1d.tensor, offset=ap1d.offset,
                       ap=[[0, p]] + list(ap1d.ap))

    with tile.TileContext(nc) as tc, ExitStack() as ctx:
        const = ctx.enter_context(tc.tile_pool(name="const", bufs=1))
        persist = ctx.enter_context(tc.tile_pool(name="persist", bufs=1))
        wpool = ctx.enter_context(tc.tile_pool(name="wpool", bufs=1))
        apool = ctx.enter_context(tc.tile_pool(name="apool", bufs=1))
        pat = ctx.enter_context(tc.tile_pool(name="pat", bufs=1))

        TS = nc.vector.tensor_scalar
        TT = nc.vector.tensor_tensor
        STT = nc.vector.scalar_tensor_tensor

        def tscopy(dst, srcap):
            TS(out=dst, in0=srcap, scalar1=0.0, scalar2=None, op0=OP.add)

        ident_t = const.tile([128, 128], F32)
        nc.sync.dma_start(out=ident_t[:], in_=ident)
        fc_t = const.tile([128, FH], F32)
        nc.sync.dma_start(out=fc_t[:], in_=pbcast(fcon, 128, FH))
        one_t = const.tile([128, 1], F32)
        nc.vector.memset(one_t[:], 1.0)
        b1_t = const.tile([128, KH], F32)
        nc.sync.dma_start(out=b1_t[:], in_=b1.rearrange("(m p) -> p m", p=128))
        bb1_t = const.tile([128, NBLOCKS, KH], F32)
        bb2_t = const.tile([128, NBLOCKS, KH], F32)
        bcb_t = const.tile([128, NBLOCKS, KH], F32)
        for tt_, src in ((bb1_t, bb1), (bb2_t, bb2), (bcb_t, bcb)):
            nc.sync.dma_start(out=tt_[:],
                              in_=src.rearrange("i (m p) -> p i m", p=128))

        # persistent activations / outputs
        ctx_T = [apool.tile([128, 2, bc], FP8, tag=f"ctxT{kp}",
                            name=f"ctxT{kp}") for kp in range(KH // 2)]
        x_T = apool.tile([64, bc], FP8)
        t_t = [apool.tile([128, bc], MMT, tag=f"t{k}", name=f"t{k}")
               for k in range(KH)]
        halfprod = persist.tile([128, nch, 2], F32)


        # -------- transposes of pred/ctx for the whole core, up front ------
        with tc.tile_pool(name="pst", bufs=2, space="PSUM") as pst, \
             tc.tile_pool(name="ldp", bufs=2) as ldp:
            for c in range(nch):
                ld = ldp.tile([128, C], F32, tag="ctxld", name="ctxld")
                nc.sync.dma_start(out=ld[:], in_=ctxm[c * 128:(c + 1) * 128, :])
                for k in range(KH):
                    ps = pst.tile([128, 128], F32, tag="tp", name="tp")
                    nc.tensor.transpose(ps[:], ld[:, k * 128:(k + 1) * 128],
                                        ident_t[:])
                    nc.scalar.activation(
                        out=ctx_T[k // 2][:, k % 2, c * 128:(c + 1) * 128],
                        in_=ps[:], func=ACTF.Copy)
                pld = ldp.tile([128, F], F32, tag="predld", name="predld")
                nc.sync.dma_start(out=pld[:], in_=pred[c * 128:(c + 1) * 128, :])
                ps = pst.tile([64, 128], F32, tag="tpp", name="tpp")
                nc.tensor.transpose(ps[:], pld[:], ident_t[:])
                nc.scalar.activation(out=x_T[:, c * 128:(c + 1) * 128],
                                     in_=ps[:], func=ACTF.Copy)

        # trunk weights, all resident (bf16)
        w_in_t = wpool.tile([64, H], FP8)
        nc.sync.dma_start(out=w_in_t[:], in_=w_in)
        wc_in_t = [wpool.tile([128, 2, H], FP8, tag=f"wci{kp}",
                              name=f"wci{kp}") for kp in range(KH // 2)]
        for kp in range(KH // 2):
            nc.sync.dma_start(
                out=wc_in_t[kp][:],
                in_=wc_in[kp * 256:(kp + 1) * 256, :].rearrange(
                    "(two p) h -> p two h", two=2))
        wstream = ctx.enter_context(tc.tile_pool(name="wstream", bufs=2))

        def load_block_w(i):
            wbt = {}
            for nm, src in (("wb1", wb1), ("wb2", wb2), ("wcb", wcb)):
                for kp in range(KH // 2):
                    t_ = wstream.tile([128, 2, H], FP8, tag=f"{nm}_{kp}",
                                      name=f"{nm}_{kp}")
                    nc.sync.dma_start(
                        out=t_[:],
                        in_=src[i, kp * 256:(kp + 1) * 256, :].rearrange(
                            "(two p) h -> p two h", two=2))
                    wbt[(nm, i, kp)] = t_
            return wbt
        # W_out: one feature-half at a time (ring reuse via same tags)
        wo_t = [wpool.tile([128, WOH], MMT, tag=f"wo{k}", name=f"wo{k}")
                for k in range(KH)]

        def load_wo(half):
            for k in range(KH):
                nc.sync.dma_start(
                    out=wo_t[k][:],
                    in_=w_out[k * 128:(k + 1) * 128,
                              half * WOH:(half + 1) * WOH])

        load_wo(0)

        psm = ctx.enter_context(tc.tile_pool(name="psm", bufs=2, space="PSUM"))
        spl = ctx.enter_context(tc.tile_pool(name="spl", bufs=2))
        grp = ctx.enter_context(tc.tile_pool(name="grp", bufs=1))

        def ps4():
            return psm.tile([128, 4, 512], F32, tag="ps4", name="ps4")

        # ---------------- phase-B chunk processing ----------------
        GRP = 8
        gtiles = {}

        def gt(nm):
            if nm not in gtiles:
                gtiles[nm] = grp.tile([128, GRP, FH], F32, tag=nm, name=nm)
            return gtiles[nm]

        gRall = grp.tile([128, GRP, 6, FH], F32, tag="gRall", name="gRall")

        def b_chunk(c, half, gi):
            """Output GEMM + spline front for batch chunk c, feature half."""
            csl = slice(c * 128, (c + 1) * 128)
            gX = gt("gX")
            nc.sync.dma_start(out=gX[:, gi, :],
                              in_=pred[csl, half * FH:(half + 1) * FH])
            EWH = spl.tile([128, 2, FH, NB], F32, tag="EWH", name="EWH")
            EW = EWH[:, 0]
            EH = EWH[:, 1]
            ED = spl.tile([128, FH, NB + 1], F32, tag="ED", name="ED",
                          bufs=1)
            for n in range(2):
                ps = ps4()
                for j in range(4):
                    gg = half * 8 + n * 4 + j
                    nk = KSETS[gg]
                    nsl = slice((n * 4 + j) * 364, (n * 4 + j + 1) * 364)
                    for k in range(nk):
                        nc.tensor.matmul(ps[:, j, 0:364],
                                         t_t[k][:, csl],
                                         wo_t[k][:, nsl],
                                         start=(k == 0), stop=(k == nk - 1))
                psv = bass.AP(tensor=ps[:].tensor, offset=ps[:].offset,
                              ap=[ps[:].ap[0], [512, 4], [MULT, 4], [1, MULT]])
                fsl = slice(n * 16, (n + 1) * 16)
                nc.scalar.activation(
                    out=EW[:, fsl, :].rearrange("p (a f) n -> p a f n", a=4),
                    in_=psv[:, :, :, 0:NB], func=ACTF.Exp, scale=SCALE)
                nc.scalar.activation(
                    out=EH[:, fsl, :].rearrange("p (a f) n -> p a f n", a=4),
                    in_=psv[:, :, :, NB:2 * NB], func=ACTF.Exp, scale=SCALE)
                nc.scalar.activation(
                    out=ED[:, fsl, :].rearrange("p (a f) n -> p a f n", a=4),
                    in_=psv[:, :, :, 2 * NB:MULT], func=ACTF.Exp)
            # D = softplus(ud) = ln(exp(ud) + 1), in place over ED
            D = ED
            nc.scalar.activation(out=D[:].rearrange("p f n -> p (f n)"),
                                 in_=ED[:].rearrange("p f n -> p (f n)"),
                                 func=ACTF.Ln, bias=one_t[:])
            # per-feature sums, both tables in one reduce
            Sw = spl.tile([128, 2, FH], F32, tag="Sw", name="Sw")
            nc.vector.tensor_reduce(out=Sw[:], in_=EWH[:],
                                    axis=AX.X, op=OP.add)
            CRb = spl.tile([128, 2, FH], F32, tag="CRb", name="CRb", bufs=1)
            nc.vector.reciprocal(out=CRb[:].rearrange("p a f -> p (a f)"),
                                 in_=Sw[:].rearrange("p a f -> p (a f)"))
            TS(out=CRb[:].rearrange("p a f -> p (a f)"),
               in0=CRb[:].rearrange("p a f -> p (a f)"),
               scalar1=CFREE, scalar2=None, op0=OP.mult)
            # normalized widths/heights + softplus diffs on GpSimd
            EWHn = spl.tile([128, 2, FH, NB], F32, tag="EWHn", name="EWHn",
                            bufs=1)
            TT(out=EWHn[:], in0=EWH[:],
               in1=bass.AP(tensor=CRb[:].tensor, offset=CRb[:].offset,
                           ap=list(CRb[:].ap) + [[0, NB]]), op=OP.mult)
            EWn = EWHn[:, 0]
            EHn = EWHn[:, 1]
            dd = spl.tile([128, FH, NB], F32, tag="dd", name="dd", bufs=1)
            TT(out=dd[:], in0=D[:, :, 1:NB + 1], in1=D[:, :, 0:NB],
               op=OP.subtract)
            # bin-search mask in one fused scan-compare
            xpf = spl.tile([128, FH], F32, tag="xpf", name="xpf", bufs=1)
            TT(out=xpf[:], in0=gX[:, gi, :], in1=fc_t[:], op=OP.add)
            u = spl.tile([128, FH, NB], F32, tag="u", name="u", bufs=1)
            nc.vector._custom_dve(scancmp, out=u[:], in0=EWn,
                                  in1=bcast(xpf[:], NB), s0=MIN_BIN)
            # six fused masked-MAC gathers (chained; diff at segment ends)
            Rbig = spl.tile([128, 6, FH, NB - 1], F32, tag="Rbig",
                            name="Rbig", bufs=1)
            u0 = u[:, :, 0:NB - 1]
            streams = ((EWn[:, :, 0:NB - 1], MIN_BIN),
                       (EWn[:, :, 1:NB], MIN_BIN),
                       (EHn[:, :, 0:NB - 1], MIN_BIN),
                       (EHn[:, :, 1:NB], MIN_BIN),
                       (dd[:, :, 0:NB - 1], 0.0),
                       (dd[:, :, 1:NB], 0.0))
            for i_s, (t_in1, imm) in enumerate(streams):
                nc.vector._custom_dve(scan_mac, out=Rbig[:, i_s, :, :],
                                      in0=u0, in1=t_in1, s0=imm)
            Rl6 = bass.AP(tensor=Rbig[:].tensor,
                          offset=Rbig[:].offset + NB - 2,
                          ap=[Rbig[:].ap[0], [FH * (NB - 1), 6], [NB - 1, FH]])
            tscopy(gRall[:, gi, :, :], Rl6)
            # first-element extracts (ACT, strided)
            nc.scalar.activation(
                out=gt("gEWn0")[:, gi, :],
                in_=bass.AP(tensor=EWn.tensor, offset=EWn.offset,
                            ap=[EWn.ap[0], [NB, FH]]), func=ACTF.Copy)
            nc.scalar.activation(
                out=gt("gEHn0")[:, gi, :],
                in_=bass.AP(tensor=EHn.tensor, offset=EHn.offset,
                            ap=[EHn.ap[0], [NB, FH]]), func=ACTF.Copy)
            nc.scalar.activation(
                out=gt("gD0")[:, gi, :],
                in_=bass.AP(tensor=D[:].tensor, offset=D[:].offset,
                            ap=[D[:].ap[0], [NB + 1, FH]]), func=ACTF.Copy)
            nc.scalar.activation(
                out=gt("gD1")[:, gi, :],
                in_=bass.AP(tensor=D[:].tensor, offset=D[:].offset + 1,
                            ap=[D[:].ap[0], [NB + 1, FH]]), func=ACTF.Copy)

        def b_group_chain(gidx, half):
            """Finish the spline for GRP chunks on [128, GRP, FH] tiles."""
            def g2t(nm):
                return grp.tile([128, GRP, FH], F32, tag=nm, name=nm, bufs=1)
            gX = gt("gX")
            gdall = grp.tile([128, GRP, 6, FH], F32, tag="gdall",
                             name="gdall", bufs=1)
            TT(out=gdall[:, :, :, 1:FH], in0=gRall[:, :, :, 1:FH],
               in1=gRall[:, :, :, 0:FH - 1], op=OP.subtract)
            tscopy(gdall[:, :, :, 0:1], gRall[:, :, :, 0:1])
            s1d = gdall[:, :, 0, :]   # in_cw (normalized left edge)
            s2d = gdall[:, :, 1, :]
            s3d = gdall[:, :, 2, :]   # in_ch
            s4d = gdall[:, :, 3, :]
            s5d = gdall[:, :, 4, :]   # D_idx - D_0
            s6d = gdall[:, :, 5, :]   # D_{idx+1} - D_1
            inw = g2t("inw")
            TT(out=inw[:], in0=s2d, in1=s1d, op=OP.subtract)
            STT(out=inw[:], in0=inw[:], scalar=MIN_BIN, in1=gt("gEWn0")[:],
                op0=OP.add, op1=OP.add)
            rw = g2t("rw")
            nc.vector.reciprocal(out=rw[:], in_=inw[:])
            th = g2t("th")
            TT(out=th[:], in0=gX[:], in1=s1d, op=OP.subtract)
            TT(out=th[:], in0=th[:], in1=rw[:], op=OP.mult)
            inh = g2t("inh")
            TT(out=inh[:], in0=s4d, in1=s3d, op=OP.subtract)
            STT(out=inh[:], in0=inh[:], scalar=MIN_BIN, in1=gt("gEHn0")[:],
                op0=OP.add, op1=OP.add)
            dl = g2t("dl")
            TT(out=dl[:], in0=inh[:], in1=rw[:], op=OP.mult)
            ind = g2t("ind")
            STT(out=ind[:], in0=s5d, scalar=MIN_DERIV, in1=gt("gD0")[:],
                op0=OP.add, op1=OP.add)
            indp = g2t("indp")
            STT(out=indp[:], in0=s6d, scalar=MIN_DERIV, in1=gt("gD1")[:],
                op0=OP.add, op1=OP.add)
            om = g2t("tA")
            TS(out=om[:], in0=th[:], scalar1=-1.0, scalar2=1.0,
               op0=OP.mult, op1=OP.add)
            ttv = g2t("ttv")
            TT(out=ttv[:], in0=th[:], in1=om[:], op=OP.mult)
            th2 = g2t("tA")
            TT(out=th2[:], in0=th[:], in1=th[:], op=OP.mult)
            na = g2t("na")
            TT(out=na[:], in0=dl[:], in1=th2[:], op=OP.mult)
            nb_ = g2t("tA")
            TT(out=nb_[:], in0=ind[:], in1=ttv[:], op=OP.mult)
            TT(out=na[:], in0=na[:], in1=nb_[:], op=OP.add)
            TT(out=na[:], in0=na[:], in1=inh[:], op=OP.mult)
            s1_ = g2t("s1_")
            TT(out=s1_[:], in0=ind[:], in1=indp[:], op=OP.add)
            STT(out=s1_[:], in0=dl[:], scalar=-2.0, in1=s1_[:],
                op0=OP.mult, op1=OP.add)
            TT(out=s1_[:], in0=s1_[:], in1=ttv[:], op=OP.mult)
            TT(out=s1_[:], in0=s1_[:], in1=dl[:], op=OP.add)
            rden = g2t("tA")
            nc.vector.reciprocal(out=rden[:], in_=s1_[:])
            cdf = na
            TT(out=cdf[:], in0=na[:], in1=rden[:], op=OP.mult)
            TT(out=cdf[:], in0=cdf[:], in1=s3d, op=OP.add)
            hp = halfprod[:, gidx * GRP:(gidx + 1) * GRP, half:half + 1]
            nc.vector.tensor_reduce(
                out=hp.rearrange("p g h -> p (g h)"), in_=cdf[:],
                axis=AX.X, op=OP.mult)

        # ---------------- phase A: MADE trunk (serial) ----------------
        DR = mybir.MatmulPerfMode.DoubleRow
        for s in range(nslices):
            bsl = slice(s * 512, (s + 1) * 512)
            ps = ps4()
            for m in range(KH):
                msl = slice(m * 128, (m + 1) * 128)
                nc.tensor.matmul(ps[:, m, :], w_in_t[:, msl],
                                 x_T[:, bsl], start=True, stop=False)
                for kp in range(KH // 2):
                    nc.tensor.matmul(ps[:, m, :],
                                     wc_in_t[kp][:, :, msl],
                                     ctx_T[kp][:, :, bsl],
                                     start=False, stop=(kp == KH // 2 - 1),
                                     perf_mode=DR)
            for m in range(KH):
                nc.scalar.activation(out=t_t[m][:, bsl], in_=ps[:, m, :],
                                     func=ACTF.Identity, bias=b1_t[:, m:m + 1])
        # residual blocks (lower-triangular wb1/wb2 blocks are zero)
        for i in range(NBLOCKS):
            wbt = load_block_w(i)
            for s in range(nslices):
                bsl = slice(s * 512, (s + 1) * 512)
                h1t = pat.tile([128, KH, 512], FP8, tag="h1t", name="h1t")
                for k in range(KH):
                    nc.scalar.activation(out=h1t[:, k, :], in_=t_t[k][:, bsl],
                                         func=ACTF.Relu)
                def tri_gemm(psx, nm, ht):
                    for m in range(KH):
                        msl = slice(m * 128, (m + 1) * 128)
                        if m == 0:
                            nc.tensor.matmul(psx[:, m, :],
                                             wbt[(nm, i, 0)][:, 0, msl],
                                             ht[:, 0, :],
                                             start=True, stop=True)
                        elif m == 1:
                            nc.tensor.matmul(psx[:, m, :],
                                             wbt[(nm, i, 0)][:, :, msl],
                                             ht[:, 0:2, :],
                                             start=True, stop=True,
                                             perf_mode=DR)
                        elif m == 2:
                            nc.tensor.matmul(psx[:, m, :],
                                             wbt[(nm, i, 0)][:, :, msl],
                                             ht[:, 0:2, :],
                                             start=True, stop=False,
                                             perf_mode=DR)
                            nc.tensor.matmul(psx[:, m, :],
                                             wbt[(nm, i, 1)][:, 0, msl],
                                             ht[:, 2, :],
                                             start=False, stop=True)
                        else:
                            nc.tensor.matmul(psx[:, m, :],
                                             wbt[(nm, i, 0)][:, :, msl],
                                             ht[:, 0:2, :],
                                             start=True, stop=False,
                                             perf_mode=DR)
                            nc.tensor.matmul(psx[:, m, :],
                                             wbt[(nm, i, 1)][:, :, msl],
                                             ht[:, 2:4, :],
                                             start=False, stop=True,
                                             perf_mode=DR)

                ps1 = ps4()
                tri_gemm(ps1, "wb1", h1t)
                h2t = pat.tile([128, KH, 512], FP8, tag="h2t", name="h2t")
                for m in range(KH):
                    nc.scalar.activation(out=h2t[:, m, :], in_=ps1[:, m, :],
                                         func=ACTF.Relu,
                                         bias=bb1_t[:, i, m:m + 1])
                ps2 = ps4()
                tri_gemm(ps2, "wb2", h2t)
                ps3 = ps4()
                for m in range(KH):
                    msl = slice(m * 128, (m + 1) * 128)
                    for kp in range(KH // 2):
                        nc.tensor.matmul(ps3[:, m, :],
                                         wbt[("wcb", i, kp)][:, :, msl],
                                         ctx_T[kp][:, :, bsl],
                                         start=(kp == 0),
                                         stop=(kp == KH // 2 - 1),
                                         perf_mode=DR)
                for m in range(KH):
                    g_ = pat.tile([128, 512], F32, tag="g", name="g", bufs=2)
                    nc.scalar.activation(out=g_[:], in_=ps3[:, m, :],
                                         func=ACTF.Sigmoid,
                                         bias=bcb_t[:, i, m:m + 1])
                    v = pat.tile([128, 512], F32, tag="v", name="v", bufs=2)
                    STT(out=v[:], in0=ps2[:, m, :],
                        scalar=bb2_t[:, i, m:m + 1], in1=g_[:],
                        op0=OP.add, op1=OP.mult)
                    nc.gpsimd.tensor_tensor(out=t_t[m][:, bsl],
                                            in0=t_t[m][:, bsl], in1=v[:],
                                            op=OP.add)

        # ---------------- phase B: output GEMM + spline ----------------
        for half in range(2):
            if half == 1:
                load_wo(1)
            for c in range(nch):
                b_chunk(c, half, c % GRP)
                if c % GRP == GRP - 1:
                    b_group_chain(c // GRP, half)

        fp = persist.tile([128, nch], F32)
        nc.vector.tensor_tensor(
            out=fp[:],
            in0=halfprod[:, :, 0:1].rearrange("p c h -> p (c h)"),
            in1=halfprod[:, :, 1:2].rearrange("p c h -> p (c h)"),
            op=OP.mult)
        nc.sync.dma_start(out=out_d.rearrange("(c p) -> p c", p=128),
                          in_=fp[:])

    nc.compile()
    return nc


def _prep_shared(W_in, b_in, Wc_in, bc_in, Wb1, bb1, Wb2, bb2, Wcb, bcb,
                 W_out, b_out, mm_dt):
    m_in, m_hh, m_out, d_h = _masks()
    assert not np.any(b_out), "nonzero b_out not supported by this kernel"
    perm = np.argsort(d_h, kind="stable")
    if mm_dt == BF16:
        wdt = ml_dtypes.bfloat16
    else:
        wdt = np.float32
    f8 = ml_dtypes.float8_e4m3
    def rnd(a):
        return np.ascontiguousarray(np.asarray(a, np.float32).astype(wdt))
    def rnd8(a):
        return np.ascontiguousarray(np.asarray(a, np.float32).astype(f8))
    shared = {
        "w_in": rnd8((W_in * m_in)[:, perm]),
        "wc_in": rnd8(np.asarray(Wc_in)[:, perm]),
        "wb1": rnd8((Wb1 * m_hh[None])[:, perm][:, :, perm]),
        "wb2": rnd8((Wb2 * m_hh[None])[:, perm][:, :, perm]),
        "wcb": rnd8(np.asarray(Wcb)[:, :, perm]),
        "w_out": rnd((W_out * m_out)[perm, :]),
        "b1": np.ascontiguousarray((b_in + bc_in).astype(np.float32)[perm]),
        "bb1": np.ascontiguousarray(np.asarray(bb1, np.float32)[:, perm]),
        "bb2": np.ascontiguousarray(np.asarray(bb2, np.float32)[:, perm]),
        "bcb": np.ascontiguousarray(np.asarray(bcb, np.float32)[:, perm]),
        "ident": np.eye(128, dtype=np.float32),
        "fcon": np.arange(FH, dtype=np.float32),
    }
    return shared


def kernel(predicates, contexts, W_in, b_in, Wc_in, bc_in, Wb1, bb1, Wb2, bb2,
           Wcb, bcb, W_out, b_out):
    global LAST_RESULTS
    predicates = np.asarray(predicates, dtype=np.float32)
    contexts = np.asarray(contexts, dtype=np.float32)
    bc = predicates.shape[0] // NCORES
    key = (bc, MM_DT)
    if key not in _CACHE:
        _CACHE[key] = _build(bc, MM_DT)
    nc = _CACHE[key]
    shared = _prep_shared(W_in, b_in, Wc_in, bc_in, Wb1, bb1, Wb2, bb2,
                          Wcb, bcb, W_out, b_out, MM_DT)
    in_maps = []
    for cid in range(NCORES):
        sl = slice(cid * bc, (cid + 1) * bc)
        m = dict(shared)
        m["pred"] = np.ascontiguousarray(predicates[sl])
        m["ctx"] = np.ascontiguousarray(contexts[sl])
        in_maps.append(m)
    res = run_bass_kernel_spmd(nc, in_maps, core_ids=list(range(NCORES)),
                               trace=TRACE)
    LAST_RESULTS = res
    return np.concatenate([res.results[i]["out"] for i in range(NCORES)])
